# revision 10
# baseline (speedup 1.0000x reference)
"""Trainium2 Bass kernel for C3Net/SchNet-style interaction block.

Reference computation (per molecule b, atom n, neighbor slot m):
  Wfil = ssp(f_ij @ W_f1 + b_f1) @ W_f2 + b_f2, masked
  y    = s @ W_in2f
  agg  = sum_m Wfil[b,n,m,:] * y[b, neighbors[b,n,m], :]
  v    = ssp(agg @ W_f2out + b_f2out) @ W_dense + b_dense
(ssp(x) = softplus(x) - log 2)

Strategy: data-parallel over the 32 molecules, 4 per NeuronCore (8 cores).
Host-side (numpy): shard, project s -> y, gather y by neighbor index with the
mask folded in (pure indexing / layout prep), transpose f_ij to contraction-
major layout, fold the "- log 2" shifts of both shifted-softplus activations
into the following layer's bias, fold b_f1 into the Exp activation's
per-partition bias operand.

The Activation engine is the bottleneck under the instruction cost model
(softplus needs two LUT passes, Exp then Ln(x+1); there is no direct
softplus table).  The pipeline is therefore organised to keep ACT busy
back-to-back:
  - 1024-edge chunks so h1 PSUM is double-buffered (2x2 banks) alongside
    wf (2x1 bank) and the f2out accumulator (2 bank-slots): mm1 of chunk
    k+1 overlaps Exp of chunk k, so ACT never waits on the PE.
  - Exp writes fp16 into a contiguous SBUF ring; Ln runs once per
    3-chunk group over [128, 3072] to amortise the ACT access latency.
  - mm2 + the (wf + b2')*y_nbh multiply (DVE scalar_tensor_tensor) + the
    neighbor reduction (PSUM-accumulated f2out matmuls, 8 m-runs of 128
    atoms per chunk) all trail the ACT stream with slack.
  - per-super-block second softplus (Exp into a contiguous u2 buffer,
    batched Ln per atom-half) + final dense layer emitted mid-stream and
    at the tail.
"""

import math

import numpy as np
import ml_dtypes

B, N, NN, A, S, F = 32, 256, 48, 128, 50, 128
NCORES = 8
MPC = B // NCORES            # molecules per core
ATOMS = MPC * N              # 1024 atoms per core
E = ATOMS * NN               # 49152 edges per core
SUPER = 128                  # atoms per super-block (output tile)
NSB = ATOMS // SUPER         # 8 super-blocks per core
SUB = 1024                   # edges per chunk (2 PSUM banks for h1)
MPB = SUB // SUPER           # 8 neighbor-slots (m) per chunk
NCH_SB = NN // MPB           # 6 chunks per super-block
NBLK = E // SUB              # 48 chunks per core
GRP = 3                      # chunks per Ln group
NGRP = NBLK // GRP           # 16 Ln groups

LOG2 = float(math.log(2.0))
BF16 = ml_dtypes.bfloat16

_BUILT = None


def _build_program():
    """Build the Bass/Tile program (one SPMD program, same for all 8 cores)."""
    import concourse.bacc as bacc
    import concourse.mybir as mybir
    from concourse import tile

    dt = mybir.dt
    AF = mybir.ActivationFunctionType
    ALU = mybir.AluOpType

    nc = bacc.Bacc("TRN2", target_bir_lowering=False, debug=False)

    f_pack = nc.dram_tensor("f_pack", [NBLK, S, SUB], dt.bfloat16,
                            kind="ExternalInput")
    y_pack = nc.dram_tensor("y_pack", [128, E], dt.bfloat16,
                            kind="ExternalInput")
    w1 = nc.dram_tensor("w1", [S, F], dt.bfloat16, kind="ExternalInput")
    w2 = nc.dram_tensor("w2", [F, F], dt.bfloat16, kind="ExternalInput")
    wf2o = nc.dram_tensor("wf2o", [F, A], dt.bfloat16, kind="ExternalInput")
    wd = nc.dram_tensor("wd", [A, A], dt.bfloat16, kind="ExternalInput")
    b1p = nc.dram_tensor("b1p", [F, 1], dt.float32, kind="ExternalInput")
    b2p = nc.dram_tensor("b2p", [F, 1], dt.float32, kind="ExternalInput")
    bf2o = nc.dram_tensor("bf2o", [A, 1], dt.float32, kind="ExternalInput")
    bdp = nc.dram_tensor("bdp", [A, 1], dt.float32, kind="ExternalInput")
    vout = nc.dram_tensor("v_out", [A, ATOMS], dt.float32,
                          kind="ExternalOutput")

    with tile.TileContext(nc) as tc:
        with (
            tc.tile_pool(name="wpool", bufs=1) as wp,
            tc.tile_pool(name="fpool", bufs=4) as fpl,
            tc.tile_pool(name="ypool", bufs=6) as ypl,
            tc.tile_pool(name="zpool", bufs=4) as zpl,
            tc.tile_pool(name="opool", bufs=2) as opl,
            tc.tile_pool(name="psumh", bufs=2, space="PSUM") as ph1,
            tc.tile_pool(name="psumw", bufs=2, space="PSUM") as pwf,
            tc.tile_pool(name="psumv", bufs=2, space="PSUM") as pv,
        ):
            w1t = wp.tile([S, F], dt.bfloat16)
            nc.sync.dma_start(w1t[:], w1[:])
            w2t = wp.tile([F, F], dt.bfloat16)
            nc.gpsimd.dma_start(w2t[:], w2[:])
            b1pt = wp.tile([F, 1], dt.float32)
            nc.gpsimd.dma_start(b1pt[:], b1p[:])
            b2pt = wp.tile([F, 1], dt.float32)
            nc.gpsimd.dma_start(b2pt[:], b2p[:])
            wf2ot = wp.tile([F, A], dt.bfloat16)
            nc.gpsimd.dma_start(wf2ot[:], wf2o[:])
            wdt = wp.tile([A, A], dt.bfloat16)
            nc.gpsimd.dma_start(wdt[:], wd[:])
            bf2ot = wp.tile([A, 1], dt.float32)
            nc.gpsimd.dma_start(bf2ot[:], bf2o[:])
            bdpt = wp.tile([A, 1], dt.float32)
            nc.gpsimd.dma_start(bdpt[:], bdp[:])
            # Contiguous rings so ACT can process multi-chunk spans in one
            # instruction (pool tiles are not guaranteed adjacent).
            u_ring = wp.tile([128, 3 * GRP * SUB], dt.float16)   # 3 groups
            sp_ring = wp.tile([128, 3 * GRP * SUB], dt.bfloat16)  # 3 groups
            u2all = wp.tile([A, ATOMS // SUPER * 128], dt.float32)

            def emit_dma(k):
                ft = fpl.tile([S, SUB], dt.bfloat16, tag="f", name=f"ft{k}")
                nc.sync.dma_start(ft[:], f_pack[k])
                yt = ypl.tile([128, SUB], dt.bfloat16, tag="y", name=f"yt{k}")
                nc.gpsimd.dma_start(
                    yt[:], y_pack[:, k * SUB:(k + 1) * SUB])
                return ft, yt

            def emit_mm1(k, ft):
                h1 = ph1.tile([128, SUB], dt.float32, tag="h1",
                              name=f"h1_{k}")
                nc.tensor.matmul(h1[:, 0:512], w1t[:], ft[:, 0:512],
                                 start=True, stop=True)
                nc.tensor.matmul(h1[:, 512:1024], w1t[:], ft[:, 512:1024],
                                 start=True, stop=True)
                return h1

            def emit_exp(k, h1):
                lo = (k % (3 * GRP)) * SUB
                nc.scalar.activation(u_ring[:, lo:lo + SUB], h1[:], AF.Exp,
                                     bias=b1pt[:])

            def emit_ln(g):
                lo = (g % 3) * (GRP * SUB)
                nc.scalar.activation(sp_ring[:, lo:lo + GRP * SUB],
                                     u_ring[:, lo:lo + GRP * SUB],
                                     AF.Ln, bias=1.0)

            def emit_mm2_stt_f2out(k, yt, v1w):
                lo = (k % (3 * GRP)) * SUB
                for j in range(2):
                    wf = pwf.tile([128, 512], dt.float32, tag="wf",
                                  name=f"wf{k}_{j}")
                    nc.tensor.matmul(
                        wf[:], w2t[:],
                        sp_ring[:, lo + j * 512:lo + (j + 1) * 512],
                        start=True, stop=True)
                    z = zpl.tile([128, 512], dt.bfloat16, tag="z",
                                 name=f"z{k}_{j}")
                    nc.vector.scalar_tensor_tensor(
                        z[:], wf[:], b2pt[:], yt[:, j * 512:(j + 1) * 512],
                        op0=ALU.add, op1=ALU.mult)
                    for r in range(4):
                        mr = (k % NCH_SB) * MPB + j * 4 + r
                        nc.tensor.matmul(v1w[:], wf2ot[:],
                                         z[:, r * 128:(r + 1) * 128],
                                         start=(mr == 0), stop=(mr == NN - 1))

            def emit_ssp2_exp(s, v1w):
                nc.scalar.activation(u2all[:, s * 128:(s + 1) * 128],
                                     v1w[:], AF.Exp, bias=bf2ot[:])

            def emit_final(half):
                """Ln + dense layer + store for one 512-atom half."""
                lo = half * 512
                v1sp = opl.tile([A, 512], dt.bfloat16, tag="v1sp",
                                name=f"v1sp{half}")
                nc.scalar.activation(v1sp[:], u2all[:, lo:lo + 512],
                                     AF.Ln, bias=1.0)
                vps = pwf.tile([128, 512], dt.float32, tag="wf",
                               name=f"vps{half}")
                nc.tensor.matmul(vps[:], wdt[:], v1sp[:],
                                 start=True, stop=True)
                ot = opl.tile([A, 512], dt.float32, tag="o", name=f"ot{half}")
                nc.vector.tensor_scalar_add(ot[:], vps[:], bdpt[:])
                nc.sync.dma_start(vout[:, lo:lo + 512], ot[:])

            # ---- software pipeline ----------------------------------------
            # ACT program order (the bottleneck engine) is kept dense:
            #   Exp_0..2, Ln_0, Exp_3..5, Ln_1, ... with per-super-block
            #   Exp2 and half-batch Ln2 slotted in a few chunks late so
            #   their PE/DVE-side dependencies are already resolved.
            PRE = 2            # mm1/dma run-ahead (bounded by h1/ft bufs)
            h1_of = {}         # k -> h1 psum tile (until Exp)
            yt_of = {}         # k -> y tile (until its group's STT)
            v1w_of = {}        # sb -> psum accumulator
            pend_ssp2 = []     # [(due_k, sb)]
            pend_final = []    # [(due_k, half)]

            for k in range(PRE):
                ft, yt = emit_dma(k)
                yt_of[k] = yt
                h1_of[k] = emit_mm1(k, ft)

            for k in range(NBLK):
                sb = k // NCH_SB
                if k % NCH_SB == 0:
                    v1w_of[sb] = pv.tile([A, 128], dt.float32, tag="v1",
                                         name=f"v1w{sb}")
                emit_exp(k, h1_of.pop(k))
                if k + PRE < NBLK:
                    ft2, yt2 = emit_dma(k + PRE)
                    yt_of[k + PRE] = yt2
                    h1_of[k + PRE] = emit_mm1(k + PRE, ft2)
                # due ssp2 / final work rides the ACT stream here
                while pend_ssp2 and pend_ssp2[0][0] <= k:
                    _, s = pend_ssp2.pop(0)
                    emit_ssp2_exp(s, v1w_of.pop(s))
                while pend_final and pend_final[0][0] <= k:
                    emit_final(pend_final.pop(0)[1])
                if k % GRP == GRP - 1:
                    g = k // GRP
                    emit_ln(g)
                    for kk in range(g * GRP, (g + 1) * GRP):
                        sb2 = kk // NCH_SB
                        emit_mm2_stt_f2out(kk, yt_of.pop(kk), v1w_of[sb2])
                        if kk % NCH_SB == NCH_SB - 1:
                            pend_ssp2.append((kk + 3, sb2))
                            if sb2 == 3:
                                pend_final.append((kk + 5, 0))

            # drain
            while pend_ssp2:
                _, s = pend_ssp2.pop(0)
                emit_ssp2_exp(s, v1w_of.pop(s))
            emit_final(1)

    nc.finalize()
    return nc


def _get_program():
    global _BUILT
    if _BUILT is None:
        _BUILT = _build_program()
    return _BUILT


def kernel(s, neighbor_mask, neighbors, f_ij,
           W_f1, b_f1, W_f2, b_f2, W_in2f, W_f2out, b_f2out, W_dense,
           b_dense):
    s = np.asarray(s, np.float32)
    neighbor_mask = np.asarray(neighbor_mask, np.float32)
    neighbors = np.asarray(neighbors)
    f_ij = np.asarray(f_ij, np.float32)
    W_f1 = np.asarray(W_f1, np.float32)
    b_f1 = np.asarray(b_f1, np.float32)
    W_f2 = np.asarray(W_f2, np.float32)
    b_f2 = np.asarray(b_f2, np.float32)
    W_in2f = np.asarray(W_in2f, np.float32)
    W_f2out = np.asarray(W_f2out, np.float32)
    b_f2out = np.asarray(b_f2out, np.float32)
    W_dense = np.asarray(W_dense, np.float32)
    b_dense = np.asarray(b_dense, np.float32)

    # Host prep: in2f projection + neighbor gather (indexing) + layout,
    # vectorized across all 8 per-core shards at once.
    y_all = s @ W_in2f                                     # [B, N, F]
    y_nbh = y_all[np.arange(B)[:, None, None], neighbors]  # [B, N, NN, F]
    y_nbh *= neighbor_mask[..., None]

    w1_b = W_f1.astype(BF16)                               # [50, 128]
    w2_b = W_f2.astype(BF16)
    wf2o_b = W_f2out.astype(BF16)
    wd_b = W_dense.astype(BF16)
    b1p = b_f1.astype(np.float32).reshape(F, 1)
    b2p = (b_f2 - LOG2 * W_f2.sum(axis=0)).astype(np.float32).reshape(F, 1)
    bf2o = b_f2out.astype(np.float32).reshape(A, 1)
    bdp = (b_dense - LOG2 * W_dense.sum(axis=0)).astype(
        np.float32).reshape(A, 1)

    # Edge order per core: (super-block, m, atom-in-super); chunk k covers
    # m-slots (k % 6)*8 .. +8 of super-block k // 6.
    f8 = (f_ij.reshape(NCORES, NSB, SUPER, NN, S)
          .transpose(0, 1, 3, 2, 4))                       # [8,NSB,NN,128,S]
    f_pack8 = np.ascontiguousarray(
        f8.reshape(NCORES, NSB, NCH_SB, MPB, SUPER, S)
        .transpose(0, 1, 2, 5, 3, 4)
        .reshape(NCORES, NBLK, S, SUB)).astype(BF16)

    y8 = (y_nbh.reshape(NCORES, NSB, SUPER, NN, F)
          .transpose(0, 1, 3, 2, 4).reshape(NCORES, E, F).astype(BF16))
    y_pack8 = np.ascontiguousarray(y8.transpose(0, 2, 1))  # [8, 128, E]

    in_maps = []
    for c in range(NCORES):
        in_maps.append({
            "f_pack": f_pack8[c],
            "y_pack": y_pack8[c],
            "w1": w1_b,
            "w2": w2_b,
            "wf2o": wf2o_b,
            "wd": wd_b,
            "b1p": b1p,
            "b2p": b2p,
            "bf2o": bf2o,
            "bdp": bdp,
        })

    from concourse.bass_utils import run_bass_kernel_spmd

    nc = _get_program()
    res = run_bass_kernel_spmd(nc, in_maps, list(range(NCORES)))

    out = np.empty((B, N, A), np.float32)
    for c in range(NCORES):
        v_c = res.results[c]["v_out"]                    # [A, ATOMS]
        out[c * MPC:(c + 1) * MPC] = np.ascontiguousarray(
            v_c.T).reshape(MPC, N, A)
    return out
